# revision 5
# baseline (speedup 1.0000x reference)
"""Trainium2 Bass kernel for a dense transformer decoder layer.

Tensor-parallel across 8 NeuronCores:
  - heads: 2 per core (of 16), ff channels: 1024 per core (of 8192)
  - W_in rows / W_out cols sharded accordingly; ReduceScatter(add) of the
    partial outputs at the end; host concatenates the 8 shards.

Per-core dataflow (token chunks of TC):
  stats (token-major x) -> s = rsqrt(mean(x^2)+eps) -> DRAM round-trip for a
  partition broadcast; RMSNorm is folded into the matmul eviction
  (t = s * (W @ x~)) with norm_w folded into W on the host and the
  normed_ages overwrite handled by patching the last two hid rows of x~ with
  a12 * rms.  W_in matmul produces q/k transposed ([hd, tok]; rope applied
  via a pairwise-swap matmul on the PE + two multiplies), v in token-major
  form ([tok, hd]) via a second matmul orientation, and the swiglu branch.
  Causal attention runs with k-token-major score tiles, exp without
  max-subtraction (scores are O(5) here so fp32/bf16 exp is safe), a
  multiplicative causal mask on diagonal blocks, and the softmax denominator
  picked up for free through an appended ones-column on v.  The combined
  [ff|attn] activations feed the W_out matmul in token-major form, written to
  DRAM and reduce-scattered.
"""

import os
import sys

for _p in ("/opt/trn_rl_repo", "/opt/pypackages"):
    if _p not in sys.path:
        sys.path.insert(0, _p)

import numpy as np
import ml_dtypes

BF16 = ml_dtypes.bfloat16

# Model dims (fixed by the problem)
T_FULL = 4096
HID = 2048
NH = 16
HD = 128
INTER = 8192
EPS = 1e-6
SCALE = 1.0 / float(np.sqrt(np.float32(HD)))

NCORES = 8
HPC = NH // NCORES          # heads per core = 2
FPC = INTER // NCORES       # ff channels per core = 1024
NFF = FPC // 128            # ff m-tiles per core (per g1/g2) = 8
NCOMB = NFF + HPC           # comb k-tiles: ff + one per head = 10
KH = HID // 128             # hid k-tiles = 16


def _build_nc(T, TC):
    import concourse.bass as bass
    import concourse.tile as tile
    from concourse import bacc, mybir

    f32 = mybir.dt.float32
    bf16 = mybir.dt.bfloat16
    AF = mybir.ActivationFunctionType
    X = mybir.AxisListType.X

    NCHUNK = T // TC
    QC = min(512, TC)            # attention q-chunk width
    NQ = TC // QC                # q-chunks per token chunk
    NB = QC // 128               # q-subblocks per q-chunk
    NW = max(TC // 512, 1)       # 512-wide n-chunks per token chunk
    WN = min(512, TC)            # n-chunk width for W_in matmul
    NT = TC // 128               # token subtiles per chunk
    NO = HID // 512              # output col chunks = 4
    JT = T // 128                # total k-blocks (tok tiles) over full T

    nc = bacc.Bacc("TRN2", target_bir_lowering=False, debug=False,
                   num_devices=NCORES)

    # ---- DRAM parameters -------------------------------------------------
    xT_d = nc.dram_tensor("xt", [HID, T], bf16, kind="ExternalInput").ap()
    xtok_d = nc.dram_tensor("xtok", [T, HID], bf16, kind="ExternalInput").ap()
    win_d = nc.dram_tensor("w_in_t", [2 * NFF + 2 * HPC, KH, 128, 128], bf16,
                           kind="ExternalInput").ap()
    wv_d = nc.dram_tensor("w_v_t", [KH, 128, HPC * 128], bf16,
                          kind="ExternalInput").ap()
    wo_d = nc.dram_tensor("w_out_t", [NCOMB, 128, HID], bf16,
                          kind="ExternalInput").ap()
    cos_d = nc.dram_tensor("cos_t", [HD, T], bf16, kind="ExternalInput").ap()
    sin_d = nc.dram_tensor("sin_t", [HD, T], bf16, kind="ExternalInput").ap()
    a12_d = nc.dram_tensor("a12", [2, T], f32, kind="ExternalInput").ap()
    swap_d = nc.dram_tensor("swapmat", [128, 128], bf16,
                            kind="ExternalInput").ap()
    mask_d = nc.dram_tensor("maskbase", [128, 896], bf16,
                            kind="ExternalInput").ap()
    ident_d = nc.dram_tensor("identity", [128, 128], bf16,
                             kind="ExternalInput").ap()
    out_d = nc.dram_tensor("out", [T // NCORES, HID], f32,
                           kind="ExternalOutput").ap()

    from contextlib import ExitStack

    with tile.TileContext(nc) as tc:
        with ExitStack() as ctx:
            const = ctx.enter_context(tc.tile_pool(name="const", bufs=1))
            kv = ctx.enter_context(tc.tile_pool(name="kv", bufs=1))
            dram = ctx.enter_context(
                tc.tile_pool(name="dram", bufs=1, space="DRAM"))
            xpool = ctx.enter_context(tc.tile_pool(name="xpool", bufs=KH + 2))
            xtokp = ctx.enter_context(tc.tile_pool(name="xtokp", bufs=3))
            statp = ctx.enter_context(tc.tile_pool(name="statp", bufs=3))
            spool = ctx.enter_context(tc.tile_pool(name="spool", bufs=2))
            stiles = ctx.enter_context(
                tc.tile_pool(name="stiles", bufs=2 * NT + 2))
            wmp = ctx.enter_context(tc.tile_pool(name="wmp", bufs=4))
            evictp = ctx.enter_context(tc.tile_pool(name="evictp", bufs=2))
            qkp = ctx.enter_context(tc.tile_pool(name="qkp", bufs=4))
            combp = ctx.enter_context(tc.tile_pool(name="combp", bufs=1))
            ppool = ctx.enter_context(tc.tile_pool(name="ppool", bufs=4))
            attnp = ctx.enter_context(tc.tile_pool(name="attnp", bufs=4))
            wop = ctx.enter_context(tc.tile_pool(name="wop", bufs=12))
            outp = ctx.enter_context(tc.tile_pool(name="outp", bufs=4))
            ps_mm = ctx.enter_context(
                tc.tile_pool(name="ps_mm", bufs=2, space="PSUM"))
            ps_misc = ps_mm
            ps_attn = ctx.enter_context(
                tc.tile_pool(name="ps_attn", bufs=4, space="PSUM"))
            ps_out = ctx.enter_context(
                tc.tile_pool(name="ps_out", bufs=2, space="PSUM"))
            # ---- constants ----------------------------------------------
            swap_sb = const.tile([128, 128], bf16, name="swap_sb")
            nc.sync.dma_start(out=swap_sb, in_=swap_d)
            mask_sb = const.tile([128, 896], bf16, name="mask_sb")
            nc.sync.dma_start(out=mask_sb, in_=mask_d)
            ident_sb = const.tile([128, 128], bf16, name="ident_sb")
            nc.sync.dma_start(out=ident_sb, in_=ident_d)
            eps_sb = const.tile([128, 1], f32, name="eps_sb")
            nc.vector.memset(eps_sb, EPS)
            # v-projection weights, resident: [128 hid-part, KH, HPC*128]
            wv_sb = const.tile([128, KH, HPC * 128], bf16, name="wv_sb")
            nc.sync.dma_start(out=wv_sb,
                              in_=wv_d.rearrange("k p j -> p k j"))

            # persistent K / V (token history)
            kT = kv.tile([128, HPC, T], bf16, name="kT")
            v_sb = kv.tile([128, HPC, JT, 129], bf16, name="v_sb")

            # DRAM scratch
            s_dram = dram.tile([T], f32, name="s_dram")
            acc = dram.tile([T, HID], f32, name="acc")
            rs_out = dram.tile([T // NCORES, HID], f32, name="rs_out")

            for c in range(NCHUNK):
                tok0 = c * TC

                # ---- stats: s = 1/sqrt(mean(x^2)+eps), per token --------
                s_tiles = []
                for tt in range(NT):
                    r0 = tok0 + tt * 128
                    xt = xtokp.tile([128, HID], bf16, tag="xtok",
                                    name=f"xt_{c}_{tt}")
                    nc.sync.dma_start(out=xt, in_=xtok_d[r0:r0 + 128, :])
                    xsq = statp.tile([128, HID], bf16, tag="xsq", bufs=2,
                                     name=f"xsq_{c}_{tt}")
                    nc.vector.tensor_mul(xsq, xt, xt)
                    ssum = statp.tile([128, 1], f32, tag="ssum",
                                      name=f"ssum_{c}_{tt}")
                    nc.vector.reduce_sum(ssum, xsq, axis=X)
                    nc.scalar.activation(ssum, ssum, AF.Sqrt, bias=eps_sb,
                                         scale=1.0 / HID)
                    s_t = stiles.tile([128, 1], f32, tag="s",
                                      name=f"s_{c}_{tt}")
                    nc.vector.reciprocal(s_t, ssum)
                    s_tiles.append(s_t)
                    nc.sync.dma_start(out=s_dram[r0:r0 + 128], in_=s_t)

                # broadcast s over partitions via DRAM round-trip
                s_bc = spool.tile([128, TC], f32, tag="sbc",
                                  name=f"sbc_{c}")
                s_slice = s_dram[tok0:tok0 + TC]
                s_b_ap = bass.AP(tensor=s_slice.tensor, offset=s_slice.offset,
                                 ap=[[0, 128]] + list(s_slice.ap))
                nc.gpsimd.dma_start(out=s_bc, in_=s_b_ap)

                # ages rows, pre-divided by s (i.e. * rms)
                a12c = spool.tile([2, TC], f32, tag="a12c", bufs=1,
                                  name=f"a12c_{c}")
                nc.sync.dma_start(out=a12c, in_=a12_d[:, tok0:tok0 + TC])
                rms2 = spool.tile([2, TC], f32, tag="rms2", bufs=1,
                                  name=f"rms2_{c}")
                nc.vector.reciprocal(rms2, s_bc[0:2, :])
                a12s = spool.tile([2, TC], bf16, tag="a12s", bufs=1,
                                  name=f"a12s_{c}")
                nc.vector.tensor_mul(a12s, a12c, rms2)

                # ---- load xT chunk (hid-major) --------------------------
                xTt = []
                for k in range(KH):
                    xk = xpool.tile([128, TC], bf16, tag="xT",
                                    name=f"xT_{c}_{k}")
                    if k == KH - 1:
                        nc.sync.dma_start(
                            out=xk[0:126, :],
                            in_=xT_d[k * 128:k * 128 + 126, tok0:tok0 + TC])
                        nc.gpsimd.dma_start(out=xk[126:128, :], in_=a12s)
                    else:
                        nc.sync.dma_start(
                            out=xk,
                            in_=xT_d[k * 128:(k + 1) * 128, tok0:tok0 + TC])
                    xTt.append(xk)

                # ---- v projection (token-major) -------------------------
                for tsub in range(NT):
                    pv = ps_mm.tile([128, HPC * 128], f32, tag="a",
                                    name=f"pv_{c}_{tsub}")
                    for k in range(KH):
                        nc.tensor.matmul(
                            pv, lhsT=xTt[k][:, tsub * 128:(tsub + 1) * 128],
                            rhs=wv_sb[:, k, :],
                            start=(k == 0), stop=(k == KH - 1))
                    j = tok0 // 128 + tsub
                    for h in range(HPC):
                        nc.vector.tensor_scalar_mul(
                            v_sb[:, h, j, 0:128], pv[:, h * 128:(h + 1) * 128],
                            s_tiles[tsub])
                        nc.vector.memset(v_sb[:, h, j, 128:129], 1.0)

                # ---- fused W_in matmul (transposed out) -----------------
                # m order: g1_0, g2_0, ..., g1_7, g2_7, qA, qB, kA, kB
                silu_t = {}
                g2_t = {}
                qk_raw = {}
                for m in range(2 * NFF + 2 * HPC):
                    wmt = wmp.tile([128, KH, 128], bf16, tag="wm",
                                   name=f"wm_{c}_{m}")
                    nc.sync.dma_start(
                        out=wmt, in_=win_d[m].rearrange("k p j -> p k j"))
                    for n in range(NW):
                        nsl = slice(n * WN, (n + 1) * WN)
                        pm = ps_mm.tile([128, WN], f32, tag="a",
                                        name=f"pm_{c}_{m}_{n}")
                        for k in range(KH):
                            nc.tensor.matmul(pm, lhsT=wmt[:, k, :],
                                             rhs=xTt[k][:, nsl],
                                             start=(k == 0),
                                             stop=(k == KH - 1))
                        if m < 2 * NFF and m % 2 == 0:      # g1
                            p = m // 2
                            t1 = evictp.tile([128, TC], bf16, tag="g1",
                                             name=f"g1_{c}_{p}")
                            if p not in silu_t:
                                silu_t[p] = (t1, evictp.tile(
                                    [128, TC], bf16, tag="silu",
                                    name=f"silu_{c}_{p}"))
                            g1t, st = silu_t[p]
                            nc.vector.tensor_mul(g1t[:, nsl], pm, s_bc[:, nsl])
                            nc.scalar.activation(st[:, nsl], g1t[:, nsl],
                                                 AF.Silu)
                        elif m < 2 * NFF:                    # g2
                            p = m // 2
                            if p not in g2_t:
                                g2_t[p] = evictp.tile([128, TC], bf16,
                                                      tag="g2",
                                                      name=f"g2_{c}_{p}")
                            g2t = g2_t[p]
                            nc.vector.tensor_mul(g2t[:, nsl], pm, s_bc[:, nsl])
                        else:                                # q or k
                            qi = m - 2 * NFF
                            if qi not in qk_raw:
                                qk_raw[qi] = qkp.tile([128, TC], bf16,
                                                      tag="qkraw",
                                                      name=f"qkraw_{c}_{qi}")
                            nc.vector.tensor_mul(qk_raw[qi][:, nsl], pm,
                                                 s_bc[:, nsl])

                # swiglu: ff = silu(g1) * g2  -> combT tiles 0..NFF-1
                combT = combp.tile([128, NCOMB, TC], bf16, tag="comb",
                                   name=f"combT_{c}")
                for p in range(NFF):
                    nc.vector.tensor_mul(combT[:, p, :], silu_t[p][1],
                                         g2_t[p])

                # ---- rope ----------------------------------------------
                cos_sb = qkp.tile([128, TC], bf16, tag="cos", bufs=2,
                                  name=f"cos_{c}")
                nc.sync.dma_start(out=cos_sb, in_=cos_d[:, tok0:tok0 + TC])
                sin_sb = qkp.tile([128, TC], bf16, tag="sin", bufs=2,
                                  name=f"sin_{c}")
                nc.sync.dma_start(out=sin_sb, in_=sin_d[:, tok0:tok0 + TC])

                qT = qkp.tile([128, HPC, TC], bf16, tag="qT", bufs=2,
                              name=f"qT_{c}")
                # (qi, destination slice): q -> qT chunk, k -> resident kT
                rope_jobs = [(h, qT[:, h, :]) for h in range(HPC)]
                rope_jobs += [(HPC + h, kT[:, h, tok0:tok0 + TC])
                              for h in range(HPC)]
                for qi, dst in rope_jobs:
                    src = qk_raw[qi]
                    for n in range(NW):
                        nsl = slice(n * WN, (n + 1) * WN)
                        psw = ps_misc.tile([128, WN], f32, tag="a",
                                           name=f"psw_{c}_{qi}_{n}")
                        nc.tensor.matmul(psw, lhsT=swap_sb, rhs=src[:, nsl],
                                         start=True, stop=True)
                        rt1 = qkp.tile([128, WN], bf16, tag="rt1", bufs=2,
                                       name=f"rt1_{c}_{qi}_{n}")
                        nc.vector.tensor_mul(rt1, psw, sin_sb[:, nsl])
                        rt2 = qkp.tile([128, WN], bf16, tag="rt2", bufs=2,
                                       name=f"rt2_{c}_{qi}_{n}")
                        nc.vector.tensor_mul(rt2, src[:, nsl], cos_sb[:, nsl])
                        nc.vector.tensor_add(dst[:, nsl], rt1, rt2)

                # ---- causal attention ----------------------------------
                for qc in range(NQ):
                    q0 = tok0 + qc * QC
                    kmax = (q0 + QC) // 128
                    for h in range(HPC):
                        pa = [ps_attn.tile([128, 129], f32, tag="attn",
                                           name=f"pa_{c}_{qc}_{h}_{i}")
                              for i in range(NB)]
                        for j in range(kmax):
                            psc = ps_misc.tile([128, QC], f32, tag="a",
                                               name=f"psc_{c}_{qc}_{h}_{j}")
                            nc.tensor.matmul(
                                psc, lhsT=kT[:, h, j * 128:(j + 1) * 128],
                                rhs=qT[:, h, qc * QC:(qc + 1) * QC],
                                start=True, stop=True)
                            pT = ppool.tile([128, QC], bf16, tag="p",
                                            name=f"pT_{c}_{qc}_{h}_{j}")
                            nc.scalar.activation(pT, psc, AF.Exp, scale=SCALE)
                            D = j * 128 - q0
                            if D >= 0:
                                nc.vector.tensor_mul(
                                    pT, pT, mask_sb[:, 384 - D:384 - D + QC])
                            for b in range(NB):
                                nc.tensor.matmul(
                                    pa[b],
                                    lhsT=pT[:, b * 128:(b + 1) * 128],
                                    rhs=v_sb[:, h, j, :],
                                    start=(j == 0), stop=(j == kmax - 1))
                        # normalize + transpose into combT
                        for b in range(NB):
                            li = attnp.tile([128, 1], f32, tag="l",
                                            name=f"l_{c}_{qc}_{h}_{b}")
                            nc.vector.reciprocal(li, pa[b][:, 128:129])
                            at = attnp.tile([128, 128], bf16, tag="at",
                                            name=f"at_{c}_{qc}_{h}_{b}")
                            nc.vector.tensor_scalar_mul(
                                at, pa[b][:, 0:128], li)
                            ptr = ps_misc.tile([128, 128], bf16, tag="a",
                                               name=f"ptr_{c}_{qc}_{h}_{b}")
                            nc.tensor.transpose(ptr, at, ident_sb)
                            col0 = qc * QC + b * 128
                            nc.scalar.copy(
                                combT[:, NFF + h, col0:col0 + 128], ptr)

                # ---- output projection (token-major) --------------------
                for oc in range(NO):
                    wots = []
                    for kc in range(NCOMB):
                        wot = wop.tile([128, 512], bf16, tag="wo",
                                       name=f"wo_{c}_{oc}_{kc}")
                        nc.sync.dma_start(
                            out=wot, in_=wo_d[kc, :, oc * 512:(oc + 1) * 512])
                        wots.append(wot)
                    for tsub in range(NT):
                        po = ps_out.tile([128, 512], f32, tag="out",
                                         name=f"po_{c}_{oc}_{tsub}")
                        for kc in range(NCOMB):
                            nc.tensor.matmul(
                                po,
                                lhsT=combT[:, kc,
                                           tsub * 128:(tsub + 1) * 128],
                                rhs=wots[kc],
                                start=(kc == 0), stop=(kc == NCOMB - 1))
                        ost = outp.tile([128, 512], f32, tag="ost",
                                        name=f"ost_{c}_{oc}_{tsub}")
                        nc.scalar.copy(ost, po)
                        r0 = tok0 + tsub * 128
                        nc.sync.dma_start(
                            out=acc[r0:r0 + 128, oc * 512:(oc + 1) * 512],
                            in_=ost)

            # ---- reduce-scatter + output -------------------------------
            from concourse import mybir as _mybir
            nc.gpsimd.collective_compute(
                "ReduceScatter",
                _mybir.AluOpType.add,
                replica_groups=[list(range(NCORES))],
                ins=[acc.opt()],
                outs=[rs_out.opt()],
            )
            nc.sync.dma_start(out=out_d, in_=rs_out[:, :])

    nc.compile()
    return nc


def _prep_in_maps(x, normed_ages, sin, cos, norm_w, W_in, W_out):
    """Shard + preprocess inputs into per-core in_maps (numpy only)."""
    T = x.shape[0]
    xT_bf = np.ascontiguousarray(x.T).astype(BF16)
    xtok_bf = x.astype(BF16)
    cos_t = np.ascontiguousarray(cos.reshape(T, HD).T).astype(BF16)
    sin_t = np.ascontiguousarray(sin.reshape(T, HD).T).astype(BF16)
    a12 = np.stack([normed_ages, normed_ages * normed_ages]).astype(np.float32)

    sw = np.zeros((128, 128), np.float32)
    idx = np.arange(0, 128, 2)
    sw[idx + 1, idx] = -1.0   # lhsT[2i+1, 2i] = -1
    sw[idx, idx + 1] = 1.0    # lhsT[2i, 2i+1] = +1
    swapmat = sw.astype(BF16)

    maskbase = (np.arange(896)[None, :] - 384 >=
                np.arange(128)[:, None]).astype(BF16)
    identity = np.eye(128, dtype=np.float32).astype(BF16)

    # norm_w folded into W_in except the last two hid columns (the
    # normed_ages overwrite bypasses the norm weight).
    def fold(wrows):
        w = wrows * norm_w[None, :]
        w[:, HID - 2:] = wrows[:, HID - 2:]
        return w

    q_base = 2 * INTER
    k_base = 2 * INTER + HID
    v_base = 2 * INTER + 2 * HID

    in_maps = []
    for core in range(NCORES):
        f0 = FPC * core
        h0 = HPC * core
        rows = []
        for p in range(NFF):
            rows.append(W_in[f0 + p * 128: f0 + (p + 1) * 128])           # g1_p
            rows.append(W_in[INTER + f0 + p * 128:
                             INTER + f0 + (p + 1) * 128])                 # g2_p
        for h in range(HPC):
            rows.append(W_in[q_base + (h0 + h) * HD:
                             q_base + (h0 + h + 1) * HD])                 # q
        for h in range(HPC):
            rows.append(W_in[k_base + (h0 + h) * HD:
                             k_base + (h0 + h + 1) * HD])                 # k
        w_used = fold(np.concatenate(rows, axis=0))                       # [2560, HID]
        nm = 2 * NFF + 2 * HPC
        w_in_t = np.ascontiguousarray(
            w_used.reshape(nm, 128, KH, 128).transpose(0, 2, 3, 1)
        ).astype(BF16)

        wv = fold(W_in[v_base + h0 * HD: v_base + (h0 + HPC) * HD])       # [256, HID]
        w_v_t = np.ascontiguousarray(
            wv.reshape(HPC * 128, KH, 128).transpose(1, 2, 0)).astype(BF16)

        # W_out columns in comb order: ff block, then attn heads
        cols = list(range(HID + f0, HID + f0 + FPC))
        for h in range(HPC):
            cols += list(range((h0 + h) * HD, (h0 + h + 1) * HD))
        w_o_loc_t = np.ascontiguousarray(W_out[:, cols].T)                # [1280, HID]
        w_out_t = np.ascontiguousarray(
            w_o_loc_t.reshape(NCOMB, 128, HID)).astype(BF16)

        in_maps.append({
            "xt": xT_bf, "xtok": xtok_bf,
            "w_in_t": w_in_t, "w_v_t": w_v_t, "w_out_t": w_out_t,
            "cos_t": cos_t, "sin_t": sin_t, "a12": a12,
            "swapmat": swapmat, "maskbase": maskbase, "identity": identity,
        })
    return in_maps


_NC_CACHE = {}


def get_nc(T=T_FULL, TC=512):
    key = (T, TC)
    if key not in _NC_CACHE:
        _NC_CACHE[key] = _build_nc(T, TC)
    return _NC_CACHE[key]


def run(x, normed_ages, sin, cos, norm_w, W_in, W_out, T=T_FULL, TC=512,
        trace=False):
    from concourse.bass_utils import run_bass_kernel_spmd
    nc = get_nc(T, TC)
    in_maps = _prep_in_maps(x, normed_ages, sin, cos, norm_w, W_in, W_out)
    res = run_bass_kernel_spmd(nc, in_maps, list(range(NCORES)), trace=trace)
    out = np.concatenate([res.results[i]["out"] for i in range(NCORES)],
                         axis=0)
    return out.astype(np.float32), res


def kernel(x, normed_ages, sin, cos, norm_w, W_in, W_out):
    out, _ = run(x, normed_ages, sin, cos, norm_w, W_in, W_out)
    return out


# revision 8
# speedup vs baseline: 1.1660x; 1.1660x over previous
"""Trainium2 Bass kernel for a dense transformer decoder layer.

Tensor-parallel across 8 NeuronCores:
  - heads: 2 per core (of 16), ff channels: 1024 per core (of 8192)
  - W_in rows / W_out cols sharded accordingly; ReduceScatter(add) of the
    partial outputs at the end; host concatenates the 8 shards.

Per-core dataflow (token chunks of TC):
  stats (token-major x) -> s = rsqrt(mean(x^2)+eps) -> DRAM round-trip for a
  partition broadcast; RMSNorm is folded into the matmul eviction
  (t = s * (W @ x~)) with norm_w folded into W on the host and the
  normed_ages overwrite handled by patching the last two hid rows of x~ with
  a12 * rms.  W_in matmul produces q/k transposed ([hd, tok]; rope applied
  via a pairwise-swap matmul on the PE + two multiplies), v in token-major
  form ([tok, hd]) via a second matmul orientation, and the swiglu branch.
  Causal attention runs with k-token-major score tiles, exp without
  max-subtraction (scores are O(5) here so fp32/bf16 exp is safe), a
  multiplicative causal mask on diagonal blocks, and the softmax denominator
  picked up for free through an appended ones-column on v.  The combined
  [ff|attn] activations feed the W_out matmul in token-major form, written to
  DRAM and reduce-scattered.
"""

import os
import sys

for _p in ("/opt/trn_rl_repo", "/opt/pypackages"):
    if _p not in sys.path:
        sys.path.insert(0, _p)

import numpy as np
import ml_dtypes

BF16 = ml_dtypes.bfloat16

# Model dims (fixed by the problem)
T_FULL = 4096
HID = 2048
NH = 16
HD = 128
INTER = 8192
EPS = 1e-6
SCALE = 1.0 / float(np.sqrt(np.float32(HD)))

NCORES = 8
HPC = NH // NCORES          # heads per core = 2
FPC = INTER // NCORES       # ff channels per core = 1024
NFF = FPC // 128            # ff m-tiles per core (per g1/g2) = 8
NCOMB = NFF + HPC           # comb k-tiles: ff + one per head = 10
KH = HID // 128             # hid k-tiles = 16


def _build_nc(T, TC):
    import concourse.bass as bass
    import concourse.tile as tile
    from concourse import bacc, mybir

    f32 = mybir.dt.float32
    bf16 = mybir.dt.bfloat16
    AF = mybir.ActivationFunctionType
    X = mybir.AxisListType.X

    NCHUNK = T // TC
    QC = min(512, TC)            # attention q-chunk width
    NQ = TC // QC                # q-chunks per token chunk
    NB = QC // 128               # q-subblocks per q-chunk
    NW = max(TC // 512, 1)       # 512-wide n-chunks per token chunk
    WN = min(512, TC)            # n-chunk width for W_in matmul
    NT = TC // 128               # token subtiles per chunk
    NO = HID // 512              # output col chunks = 4
    JT = T // 128                # total k-blocks (tok tiles) over full T

    nc = bacc.Bacc("TRN2", target_bir_lowering=False, debug=False,
                   num_devices=NCORES)

    # ---- DRAM parameters -------------------------------------------------
    xT_d = nc.dram_tensor("xt", [HID, T], bf16, kind="ExternalInput").ap()
    xtok_d = nc.dram_tensor("xtok", [T, HID], bf16, kind="ExternalInput").ap()
    win_d = nc.dram_tensor("w_in_t", [2 * NFF + 2 * HPC, 128, KH, 128], bf16,
                           kind="ExternalInput").ap()
    wv_d = nc.dram_tensor("w_v_t", [128, KH, HPC * 128], bf16,
                          kind="ExternalInput").ap()
    wo_d = nc.dram_tensor("w_out_t", [NO, 128, NCOMB, 512], bf16,
                          kind="ExternalInput").ap()
    cos_d = nc.dram_tensor("cos_t", [HD, T], bf16, kind="ExternalInput").ap()
    sin_d = nc.dram_tensor("sin_t", [HD, T], bf16, kind="ExternalInput").ap()
    a12_d = nc.dram_tensor("a12", [2, T], f32, kind="ExternalInput").ap()
    swap_d = nc.dram_tensor("swapmat", [128, 128], bf16,
                            kind="ExternalInput").ap()
    mask_d = nc.dram_tensor("maskbase", [128, 896], bf16,
                            kind="ExternalInput").ap()
    ident_d = nc.dram_tensor("identity", [128, 128], bf16,
                             kind="ExternalInput").ap()
    out_d = nc.dram_tensor("out", [NCHUNK, TC // NCORES, HID], f32,
                           kind="ExternalOutput").ap()

    from contextlib import ExitStack

    with tile.TileContext(nc) as tc:
        with ExitStack() as ctx:
            const = ctx.enter_context(tc.tile_pool(name="const", bufs=1))
            kv = ctx.enter_context(tc.tile_pool(name="kv", bufs=1))
            dram = ctx.enter_context(
                tc.tile_pool(name="dram", bufs=1, space="DRAM"))
            xpool = ctx.enter_context(tc.tile_pool(name="xpool", bufs=KH + 2))
            xtokp = ctx.enter_context(tc.tile_pool(name="xtokp", bufs=3))
            statp = ctx.enter_context(tc.tile_pool(name="statp", bufs=3))
            spool = ctx.enter_context(tc.tile_pool(name="spool", bufs=2))
            stiles = ctx.enter_context(
                tc.tile_pool(name="stiles", bufs=2 * NT + 2))
            wmp = ctx.enter_context(tc.tile_pool(name="wmp", bufs=4))
            evictp = ctx.enter_context(tc.tile_pool(name="evictp", bufs=2))
            qkp = ctx.enter_context(tc.tile_pool(name="qkp", bufs=4))
            combp = ctx.enter_context(tc.tile_pool(name="combp", bufs=1))
            ppool = ctx.enter_context(tc.tile_pool(name="ppool", bufs=4))
            attnp = ctx.enter_context(tc.tile_pool(name="attnp", bufs=4))
            wop = ctx.enter_context(tc.tile_pool(name="wop", bufs=12))
            outp = ctx.enter_context(tc.tile_pool(name="outp", bufs=4))
            ps_mm = ctx.enter_context(
                tc.tile_pool(name="ps_mm", bufs=2, space="PSUM"))
            ps_misc = ps_mm
            ps_attn = ctx.enter_context(
                tc.tile_pool(name="ps_attn", bufs=4, space="PSUM"))
            ps_out = ctx.enter_context(
                tc.tile_pool(name="ps_out", bufs=2, space="PSUM"))
            # ---- constants ----------------------------------------------
            swap_sb = const.tile([128, 128], bf16, name="swap_sb")
            nc.sync.dma_start(out=swap_sb, in_=swap_d)
            mask_sb = const.tile([128, 896], bf16, name="mask_sb")
            nc.sync.dma_start(out=mask_sb, in_=mask_d)
            ident_sb = const.tile([128, 128], bf16, name="ident_sb")
            nc.sync.dma_start(out=ident_sb, in_=ident_d)
            eps_sb = const.tile([128, 1], f32, name="eps_sb")
            nc.vector.memset(eps_sb, EPS)
            # v-projection weights, resident: [128 hid-part, KH, HPC*128]
            wv_sb = const.tile([128, KH, HPC * 128], bf16, name="wv_sb")
            nc.sync.dma_start(out=wv_sb, in_=wv_d)

            # persistent K / V (token history)
            kT = kv.tile([128, HPC, T], bf16, name="kT")
            v_sb = kv.tile([128, HPC, JT, 129], bf16, name="v_sb")

            # DRAM scratch
            s_dram = dram.tile([T], f32, name="s_dram")
            acc = dram.tile([T, HID], f32, name="acc")
            rs_out = dram.tile([NCHUNK, TC // NCORES, HID], f32,
                               name="rs_out")

            for c in range(NCHUNK):
                tok0 = c * TC

                # ---- stats: s = 1/sqrt(mean(x^2)+eps), per token --------
                s_tiles = []
                for tt in range(NT):
                    r0 = tok0 + tt * 128
                    xt = xtokp.tile([128, HID], bf16, tag="xtok",
                                    name=f"xt_{c}_{tt}")
                    nc.gpsimd.dma_start(out=xt, in_=xtok_d[r0:r0 + 128, :])
                    xsq = statp.tile([128, HID], bf16, tag="xsq", bufs=2,
                                     name=f"xsq_{c}_{tt}")
                    nc.vector.tensor_mul(xsq, xt, xt)
                    ssum = statp.tile([128, 1], f32, tag="ssum",
                                      name=f"ssum_{c}_{tt}")
                    nc.vector.reduce_sum(ssum, xsq, axis=X)
                    nc.scalar.activation(ssum, ssum, AF.Sqrt, bias=eps_sb,
                                         scale=1.0 / HID)
                    s_t = stiles.tile([128, 1], f32, tag="s",
                                      name=f"s_{c}_{tt}")
                    nc.vector.reciprocal(s_t, ssum)
                    s_tiles.append(s_t)
                    nc.gpsimd.dma_start(out=s_dram[r0:r0 + 128], in_=s_t)

                # broadcast s over partitions via DRAM round-trip
                s_bc = spool.tile([128, TC], f32, tag="sbc",
                                  name=f"sbc_{c}")
                s_slice = s_dram[tok0:tok0 + TC]
                s_b_ap = bass.AP(tensor=s_slice.tensor, offset=s_slice.offset,
                                 ap=[[0, 128]] + list(s_slice.ap))
                nc.gpsimd.dma_start(out=s_bc, in_=s_b_ap)

                # ages rows, pre-divided by s (i.e. * rms)
                a12c = spool.tile([2, TC], f32, tag="a12c", bufs=1,
                                  name=f"a12c_{c}")
                nc.gpsimd.dma_start(out=a12c, in_=a12_d[:, tok0:tok0 + TC])
                rms2 = spool.tile([2, TC], f32, tag="rms2", bufs=1,
                                  name=f"rms2_{c}")
                nc.vector.reciprocal(rms2, s_bc[0:2, :])
                a12s = spool.tile([2, TC], bf16, tag="a12s", bufs=1,
                                  name=f"a12s_{c}")
                nc.vector.tensor_mul(a12s, a12c, rms2)

                # ---- load xT chunk (hid-major) --------------------------
                xTt = []
                for k in range(KH):
                    xk = xpool.tile([128, TC], bf16, tag="xT",
                                    name=f"xT_{c}_{k}")
                    if k == KH - 1:
                        nc.gpsimd.dma_start(
                            out=xk[0:126, :],
                            in_=xT_d[k * 128:k * 128 + 126, tok0:tok0 + TC])
                        nc.gpsimd.dma_start(out=xk[126:128, :], in_=a12s)
                    else:
                        nc.gpsimd.dma_start(
                            out=xk,
                            in_=xT_d[k * 128:(k + 1) * 128, tok0:tok0 + TC])
                    xTt.append(xk)

                # ---- v projection (token-major) -------------------------
                for tsub in range(NT):
                    pv = ps_mm.tile([128, HPC * 128], f32, tag="a",
                                    name=f"pv_{c}_{tsub}")
                    for k in range(KH):
                        nc.tensor.matmul(
                            pv, lhsT=xTt[k][:, tsub * 128:(tsub + 1) * 128],
                            rhs=wv_sb[:, k, :],
                            start=(k == 0), stop=(k == KH - 1))
                    j = tok0 // 128 + tsub
                    for h in range(HPC):
                        nc.vector.tensor_scalar_mul(
                            v_sb[:, h, j, 0:128], pv[:, h * 128:(h + 1) * 128],
                            s_tiles[tsub])
                        nc.vector.memset(v_sb[:, h, j, 128:129], 1.0)

                # ---- fused W_in matmul (transposed out) -----------------
                # m order: g1_0, g2_0, ..., g1_7, g2_7, qA, qB, kA, kB
                silu_t = {}
                g2_t = {}
                qk_raw = {}
                for m in range(2 * NFF + 2 * HPC):
                    wmt = wmp.tile([128, KH, 128], bf16, tag="wm",
                                   name=f"wm_{c}_{m}")
                    nc.scalar.dma_start(out=wmt, in_=win_d[m])
                    for n in range(NW):
                        nsl = slice(n * WN, (n + 1) * WN)
                        pm = ps_mm.tile([128, WN], f32, tag="a",
                                        name=f"pm_{c}_{m}_{n}")
                        for k in range(KH):
                            nc.tensor.matmul(pm, lhsT=wmt[:, k, :],
                                             rhs=xTt[k][:, nsl],
                                             start=(k == 0),
                                             stop=(k == KH - 1))
                        if m < 2 * NFF and m % 2 == 0:      # g1
                            p = m // 2
                            t1 = evictp.tile([128, TC], bf16, tag="g1",
                                             name=f"g1_{c}_{p}")
                            if p not in silu_t:
                                silu_t[p] = (t1, evictp.tile(
                                    [128, TC], bf16, tag="silu",
                                    name=f"silu_{c}_{p}"))
                            g1t, st = silu_t[p]
                            nc.vector.tensor_mul(g1t[:, nsl], pm, s_bc[:, nsl])
                            nc.scalar.activation(st[:, nsl], g1t[:, nsl],
                                                 AF.Silu)
                        elif m < 2 * NFF:                    # g2
                            p = m // 2
                            if p not in g2_t:
                                g2_t[p] = evictp.tile([128, TC], bf16,
                                                      tag="g2",
                                                      name=f"g2_{c}_{p}")
                            g2t = g2_t[p]
                            nc.vector.tensor_mul(g2t[:, nsl], pm, s_bc[:, nsl])
                        else:                                # q or k
                            qi = m - 2 * NFF
                            if qi not in qk_raw:
                                qk_raw[qi] = qkp.tile([128, TC], bf16,
                                                      tag="qkraw",
                                                      name=f"qkraw_{c}_{qi}")
                            nc.vector.tensor_mul(qk_raw[qi][:, nsl], pm,
                                                 s_bc[:, nsl])

                # swiglu: ff = silu(g1) * g2  -> combT tiles 0..NFF-1
                combT = combp.tile([128, NCOMB, TC], bf16, tag="comb",
                                   name=f"combT_{c}")
                for p in range(NFF):
                    nc.vector.tensor_mul(combT[:, p, :], silu_t[p][1],
                                         g2_t[p])

                # ---- rope ----------------------------------------------
                cos_sb = qkp.tile([128, TC], bf16, tag="cos", bufs=2,
                                  name=f"cos_{c}")
                nc.gpsimd.dma_start(out=cos_sb, in_=cos_d[:, tok0:tok0 + TC])
                sin_sb = qkp.tile([128, TC], bf16, tag="sin", bufs=2,
                                  name=f"sin_{c}")
                nc.gpsimd.dma_start(out=sin_sb, in_=sin_d[:, tok0:tok0 + TC])

                qT = qkp.tile([128, HPC, TC], bf16, tag="qT", bufs=2,
                              name=f"qT_{c}")
                # (qi, destination slice): q -> qT chunk, k -> resident kT
                rope_jobs = [(h, qT[:, h, :]) for h in range(HPC)]
                rope_jobs += [(HPC + h, kT[:, h, tok0:tok0 + TC])
                              for h in range(HPC)]
                for qi, dst in rope_jobs:
                    src = qk_raw[qi]
                    for n in range(NW):
                        nsl = slice(n * WN, (n + 1) * WN)
                        psw = ps_misc.tile([128, WN], f32, tag="a",
                                           name=f"psw_{c}_{qi}_{n}")
                        nc.tensor.matmul(psw, lhsT=swap_sb, rhs=src[:, nsl],
                                         start=True, stop=True)
                        rt1 = qkp.tile([128, WN], bf16, tag="rt1", bufs=2,
                                       name=f"rt1_{c}_{qi}_{n}")
                        nc.vector.tensor_mul(rt1, psw, sin_sb[:, nsl])
                        rt2 = qkp.tile([128, WN], bf16, tag="rt2", bufs=2,
                                       name=f"rt2_{c}_{qi}_{n}")
                        nc.vector.tensor_mul(rt2, src[:, nsl], cos_sb[:, nsl])
                        nc.vector.tensor_add(dst[:, nsl], rt1, rt2)

                # ---- causal attention ----------------------------------
                for qc in range(NQ):
                    q0 = tok0 + qc * QC
                    kmax = (q0 + QC) // 128
                    for h in range(HPC):
                        pa = [ps_attn.tile([128, 129], f32, tag="attn",
                                           name=f"pa_{c}_{qc}_{h}_{i}")
                              for i in range(NB)]
                        for j in range(kmax):
                            psc = ps_misc.tile([128, QC], f32, tag="a",
                                               name=f"psc_{c}_{qc}_{h}_{j}")
                            nc.tensor.matmul(
                                psc, lhsT=kT[:, h, j * 128:(j + 1) * 128],
                                rhs=qT[:, h, qc * QC:(qc + 1) * QC],
                                start=True, stop=True)
                            pT = ppool.tile([128, QC], bf16, tag="p",
                                            name=f"pT_{c}_{qc}_{h}_{j}")
                            nc.scalar.activation(pT, psc, AF.Exp, scale=SCALE)
                            D = j * 128 - q0
                            if D >= 0:
                                nc.vector.tensor_mul(
                                    pT, pT, mask_sb[:, 384 - D:384 - D + QC])
                            for b in range(NB):
                                nc.tensor.matmul(
                                    pa[b],
                                    lhsT=pT[:, b * 128:(b + 1) * 128],
                                    rhs=v_sb[:, h, j, :],
                                    start=(j == 0), stop=(j == kmax - 1))
                        # normalize + transpose into combT
                        for b in range(NB):
                            li = attnp.tile([128, 1], f32, tag="l",
                                            name=f"l_{c}_{qc}_{h}_{b}")
                            nc.vector.reciprocal(li, pa[b][:, 128:129])
                            at = attnp.tile([128, 128], bf16, tag="at",
                                            name=f"at_{c}_{qc}_{h}_{b}")
                            nc.vector.tensor_scalar_mul(
                                at, pa[b][:, 0:128], li)
                            ptr = ps_misc.tile([128, 128], bf16, tag="a",
                                               name=f"ptr_{c}_{qc}_{h}_{b}")
                            nc.tensor.transpose(ptr, at, ident_sb)
                            col0 = qc * QC + b * 128
                            nc.scalar.copy(
                                combT[:, NFF + h, col0:col0 + 128], ptr)

                # ---- output projection (token-major) --------------------
                for oc in range(NO):
                    wot = wop.tile([128, NCOMB, 512], bf16, tag="wo", bufs=2,
                                   name=f"wo_{c}_{oc}")
                    nc.scalar.dma_start(out=wot, in_=wo_d[oc])
                    for tsub in range(NT):
                        po = ps_out.tile([128, 512], f32, tag="out",
                                         name=f"po_{c}_{oc}_{tsub}")
                        for kc in range(NCOMB):
                            nc.tensor.matmul(
                                po,
                                lhsT=combT[:, kc,
                                           tsub * 128:(tsub + 1) * 128],
                                rhs=wot[:, kc, :],
                                start=(kc == 0), stop=(kc == NCOMB - 1))
                        ost = outp.tile([128, 512], f32, tag="ost",
                                        name=f"ost_{c}_{oc}_{tsub}")
                        nc.vector.tensor_copy(ost, po)
                        r0 = tok0 + tsub * 128
                        nc.sync.dma_start(
                            out=acc[r0:r0 + 128, oc * 512:(oc + 1) * 512],
                            in_=ost)

                # ---- reduce-scatter this chunk's partial output ---------
                nc.gpsimd.collective_compute(
                    "ReduceScatter",
                    mybir.AluOpType.add,
                    replica_groups=[list(range(NCORES))],
                    ins=[acc[tok0:tok0 + TC, :]],
                    outs=[rs_out[c]],
                )
                nc.sync.dma_start(out=out_d[c], in_=rs_out[c])



    nc.compile()
    return nc


def _prep_in_maps(x, normed_ages, sin, cos, norm_w, W_in, W_out):
    """Shard + preprocess inputs into per-core in_maps (numpy only)."""
    T = x.shape[0]
    xT_bf = np.ascontiguousarray(x.T).astype(BF16)
    xtok_bf = x.astype(BF16)
    cos_t = np.ascontiguousarray(cos.reshape(T, HD).T).astype(BF16)
    sin_t = np.ascontiguousarray(sin.reshape(T, HD).T).astype(BF16)
    a12 = np.stack([normed_ages, normed_ages * normed_ages]).astype(np.float32)

    sw = np.zeros((128, 128), np.float32)
    idx = np.arange(0, 128, 2)
    sw[idx + 1, idx] = -1.0   # lhsT[2i+1, 2i] = -1
    sw[idx, idx + 1] = 1.0    # lhsT[2i, 2i+1] = +1
    swapmat = sw.astype(BF16)

    maskbase = (np.arange(896)[None, :] - 384 >=
                np.arange(128)[:, None]).astype(BF16)
    identity = np.eye(128, dtype=np.float32).astype(BF16)

    # norm_w folded into W_in except the last two hid columns (the
    # normed_ages overwrite bypasses the norm weight).
    def fold(wrows):
        w = wrows * norm_w[None, :]
        w[:, HID - 2:] = wrows[:, HID - 2:]
        return w

    q_base = 2 * INTER
    k_base = 2 * INTER + HID
    v_base = 2 * INTER + 2 * HID

    in_maps = []
    for core in range(NCORES):
        f0 = FPC * core
        h0 = HPC * core
        rows = []
        for p in range(NFF):
            rows.append(W_in[f0 + p * 128: f0 + (p + 1) * 128])           # g1_p
            rows.append(W_in[INTER + f0 + p * 128:
                             INTER + f0 + (p + 1) * 128])                 # g2_p
        for h in range(HPC):
            rows.append(W_in[q_base + (h0 + h) * HD:
                             q_base + (h0 + h + 1) * HD])                 # q
        for h in range(HPC):
            rows.append(W_in[k_base + (h0 + h) * HD:
                             k_base + (h0 + h + 1) * HD])                 # k
        w_used = fold(np.concatenate(rows, axis=0))                       # [2560, HID]
        nm = 2 * NFF + 2 * HPC
        # [m, p(hid-in-tile), k, j(row-in-tile)] so each partition is linear
        w_in_t = np.ascontiguousarray(
            w_used.reshape(nm, 128, KH, 128).transpose(0, 3, 2, 1)
        ).astype(BF16)

        wv = fold(W_in[v_base + h0 * HD: v_base + (h0 + HPC) * HD])       # [256, HID]
        w_v_t = np.ascontiguousarray(
            wv.reshape(HPC * 128, KH, 128).transpose(2, 1, 0)).astype(BF16)

        # W_out columns in comb order: ff block, then attn heads
        cols = list(range(HID + f0, HID + f0 + FPC))
        for h in range(HPC):
            cols += list(range((h0 + h) * HD, (h0 + h + 1) * HD))
        w_o_loc_t = np.ascontiguousarray(W_out[:, cols].T)                # [1280, HID]
        # [oc, p(c-in-tile), kc, ow] so each partition is linear per oc
        w_out_t = np.ascontiguousarray(
            w_o_loc_t.reshape(NCOMB, 128, HID // 512, 512)
            .transpose(2, 1, 0, 3)).astype(BF16)

        in_maps.append({
            "xt": xT_bf, "xtok": xtok_bf,
            "w_in_t": w_in_t, "w_v_t": w_v_t, "w_out_t": w_out_t,
            "cos_t": cos_t, "sin_t": sin_t, "a12": a12,
            "swapmat": swapmat, "maskbase": maskbase, "identity": identity,
        })
    return in_maps


_NC_CACHE = {}


def get_nc(T=T_FULL, TC=512):
    key = (T, TC)
    if key not in _NC_CACHE:
        _NC_CACHE[key] = _build_nc(T, TC)
    return _NC_CACHE[key]


def run(x, normed_ages, sin, cos, norm_w, W_in, W_out, T=T_FULL, TC=512,
        trace=False):
    from concourse.bass_utils import run_bass_kernel_spmd
    nc = get_nc(T, TC)
    in_maps = _prep_in_maps(x, normed_ages, sin, cos, norm_w, W_in, W_out)
    res = run_bass_kernel_spmd(nc, in_maps, list(range(NCORES)), trace=trace)
    # results[i]["out"][c] holds reduced rows [c*TC + i*(TC/8) : +TC/8]
    nchunk = T // TC
    seg = TC // NCORES
    out = np.empty((T, HID), np.float32)
    for i in range(NCORES):
        oi = np.asarray(res.results[i]["out"], np.float32)
        for c in range(nchunk):
            r0 = c * TC + i * seg
            out[r0:r0 + seg] = oi[c]
    return out, res


def kernel(x, normed_ages, sin, cos, norm_w, W_in, W_out):
    out, _ = run(x, normed_ages, sin, cos, norm_w, W_in, W_out)
    return out


# revision 9
# speedup vs baseline: 1.1812x; 1.0130x over previous
"""Trainium2 Bass kernel for a dense transformer decoder layer.

Tensor-parallel across 8 NeuronCores:
  - heads: 2 per core (of 16), ff channels: 1024 per core (of 8192)
  - W_in rows / W_out cols sharded accordingly; ReduceScatter(add) of the
    partial outputs at the end; host concatenates the 8 shards.

Per-core dataflow (token chunks of TC):
  stats (token-major x) -> s = rsqrt(mean(x^2)+eps) -> DRAM round-trip for a
  partition broadcast; RMSNorm is folded into the matmul eviction
  (t = s * (W @ x~)) with norm_w folded into W on the host and the
  normed_ages overwrite handled by patching the last two hid rows of x~ with
  a12 * rms.  W_in matmul produces q/k transposed ([hd, tok]; rope applied
  via a pairwise-swap matmul on the PE + two multiplies), v in token-major
  form ([tok, hd]) via a second matmul orientation, and the swiglu branch.
  Causal attention runs with k-token-major score tiles, exp without
  max-subtraction (scores are O(5) here so fp32/bf16 exp is safe), a
  multiplicative causal mask on diagonal blocks, and the softmax denominator
  picked up for free through an appended ones-column on v.  The combined
  [ff|attn] activations feed the W_out matmul in token-major form, written to
  DRAM and reduce-scattered.
"""

import os
import sys

for _p in ("/opt/trn_rl_repo", "/opt/pypackages"):
    if _p not in sys.path:
        sys.path.insert(0, _p)

import numpy as np
import ml_dtypes

BF16 = ml_dtypes.bfloat16

# Model dims (fixed by the problem)
T_FULL = 4096
HID = 2048
NH = 16
HD = 128
INTER = 8192
EPS = 1e-6
SCALE = 1.0 / float(np.sqrt(np.float32(HD)))

NCORES = 8
HPC = NH // NCORES          # heads per core = 2
FPC = INTER // NCORES       # ff channels per core = 1024
NFF = FPC // 128            # ff m-tiles per core (per g1/g2) = 8
NCOMB = NFF + HPC           # comb k-tiles: ff + one per head = 10
KH = HID // 128             # hid k-tiles = 16


def _build_nc(T, TC):
    import concourse.bass as bass
    import concourse.tile as tile
    from concourse import bacc, mybir

    f32 = mybir.dt.float32
    bf16 = mybir.dt.bfloat16
    AF = mybir.ActivationFunctionType
    X = mybir.AxisListType.X

    NCHUNK = T // TC
    QC = min(512, TC)            # attention q-chunk width
    NQ = TC // QC                # q-chunks per token chunk
    NB = QC // 128               # q-subblocks per q-chunk
    NW = max(TC // 512, 1)       # 512-wide n-chunks per token chunk
    WN = min(512, TC)            # n-chunk width for W_in matmul
    NT = TC // 128               # token subtiles per chunk
    NO = HID // 512              # output col chunks = 4
    JT = T // 128                # total k-blocks (tok tiles) over full T

    nc = bacc.Bacc("TRN2", target_bir_lowering=False, debug=False,
                   num_devices=NCORES)

    # ---- DRAM parameters -------------------------------------------------
    xT_d = nc.dram_tensor("xt", [HID, T], bf16, kind="ExternalInput").ap()
    xtok_d = nc.dram_tensor("xtok", [T, HID], bf16, kind="ExternalInput").ap()
    win_d = nc.dram_tensor("w_in_t", [2 * NFF + 2 * HPC, 128, KH, 128], bf16,
                           kind="ExternalInput").ap()
    wv_d = nc.dram_tensor("w_v_t", [128, KH, HPC * 128], bf16,
                          kind="ExternalInput").ap()
    wo_d = nc.dram_tensor("w_out_t", [NO, 128, NCOMB, 512], bf16,
                          kind="ExternalInput").ap()
    cos_d = nc.dram_tensor("cos_t", [HD, T], bf16, kind="ExternalInput").ap()
    sin_d = nc.dram_tensor("sin_t", [HD, T], bf16, kind="ExternalInput").ap()
    a12_d = nc.dram_tensor("a12", [2, T], f32, kind="ExternalInput").ap()
    swap_d = nc.dram_tensor("swapmat", [128, 128], bf16,
                            kind="ExternalInput").ap()
    mask_d = nc.dram_tensor("maskbase", [128, 896], bf16,
                            kind="ExternalInput").ap()
    ident_d = nc.dram_tensor("identity", [128, 128], bf16,
                             kind="ExternalInput").ap()
    out_d = nc.dram_tensor("out", [NCHUNK, TC // NCORES, HID], f32,
                           kind="ExternalOutput").ap()

    from contextlib import ExitStack

    with tile.TileContext(nc) as tc:
        with ExitStack() as ctx:
            const = ctx.enter_context(tc.tile_pool(name="const", bufs=1))
            kv = ctx.enter_context(tc.tile_pool(name="kv", bufs=1))
            dram = ctx.enter_context(
                tc.tile_pool(name="dram", bufs=1, space="DRAM"))
            xpool = ctx.enter_context(tc.tile_pool(name="xpool", bufs=KH + 6))
            xtokp = ctx.enter_context(tc.tile_pool(name="xtokp", bufs=3))
            statp = ctx.enter_context(tc.tile_pool(name="statp", bufs=3))
            spool = ctx.enter_context(tc.tile_pool(name="spool", bufs=2))
            stiles = ctx.enter_context(
                tc.tile_pool(name="stiles", bufs=2 * NT + 2))
            wmp = ctx.enter_context(tc.tile_pool(name="wmp", bufs=6))
            evictp = ctx.enter_context(tc.tile_pool(name="evictp", bufs=2))
            qkp = ctx.enter_context(tc.tile_pool(name="qkp", bufs=4))
            combp = ctx.enter_context(tc.tile_pool(name="combp", bufs=1))
            ppool = ctx.enter_context(tc.tile_pool(name="ppool", bufs=4))
            attnp = ctx.enter_context(tc.tile_pool(name="attnp", bufs=4))
            wop = ctx.enter_context(tc.tile_pool(name="wop", bufs=12))
            outp = ctx.enter_context(tc.tile_pool(name="outp", bufs=4))
            ps_mm = ctx.enter_context(
                tc.tile_pool(name="ps_mm", bufs=2, space="PSUM"))
            ps_misc = ps_mm
            ps_attn = ctx.enter_context(
                tc.tile_pool(name="ps_attn", bufs=4, space="PSUM"))
            ps_out = ctx.enter_context(
                tc.tile_pool(name="ps_out", bufs=2, space="PSUM"))
            # ---- constants ----------------------------------------------
            swap_sb = const.tile([128, 128], bf16, name="swap_sb")
            nc.sync.dma_start(out=swap_sb, in_=swap_d)
            mask_sb = const.tile([128, 896], bf16, name="mask_sb")
            nc.sync.dma_start(out=mask_sb, in_=mask_d)
            ident_sb = const.tile([128, 128], bf16, name="ident_sb")
            nc.sync.dma_start(out=ident_sb, in_=ident_d)
            eps_sb = const.tile([128, 1], f32, name="eps_sb")
            nc.vector.memset(eps_sb, EPS)
            # v-projection weights, resident: [128 hid-part, KH, HPC*128]
            wv_sb = const.tile([128, KH, HPC * 128], bf16, name="wv_sb")
            nc.sync.dma_start(out=wv_sb, in_=wv_d)

            # persistent K / V (token history)
            kT = kv.tile([128, HPC, T], bf16, name="kT")
            v_sb = kv.tile([128, HPC, JT, 129], bf16, name="v_sb")

            # DRAM scratch
            acc = dram.tile([T, HID], f32, name="acc")
            rs_out = dram.tile([NCHUNK, TC // NCORES, HID], f32,
                               name="rs_out")

            for c in range(NCHUNK):
                tok0 = c * TC

                # ---- stats: s = 1/sqrt(mean(x^2)+eps), per token --------
                s_dram = dram.tile([TC], f32, tag="s_dram", bufs=2,
                                   name=f"s_dram_{c}")
                s_tiles = []
                for tt in range(NT):
                    r0 = tok0 + tt * 128
                    xt = xtokp.tile([128, HID], bf16, tag="xtok",
                                    name=f"xt_{c}_{tt}")
                    nc.gpsimd.dma_start(out=xt, in_=xtok_d[r0:r0 + 128, :])
                    xsq = statp.tile([128, HID], bf16, tag="xsq", bufs=2,
                                     name=f"xsq_{c}_{tt}")
                    nc.vector.tensor_mul(xsq, xt, xt)
                    ssum = statp.tile([128, 1], f32, tag="ssum",
                                      name=f"ssum_{c}_{tt}")
                    nc.vector.reduce_sum(ssum, xsq, axis=X)
                    nc.scalar.activation(ssum, ssum, AF.Sqrt, bias=eps_sb,
                                         scale=1.0 / HID)
                    s_t = stiles.tile([128, 1], f32, tag="s",
                                      name=f"s_{c}_{tt}")
                    nc.vector.reciprocal(s_t, ssum)
                    s_tiles.append(s_t)
                    nc.gpsimd.dma_start(out=s_dram[tt * 128:(tt + 1) * 128], in_=s_t)

                # broadcast s over partitions via DRAM round-trip
                s_bc = spool.tile([128, TC], f32, tag="sbc",
                                  name=f"sbc_{c}")
                s_slice = s_dram[:]
                s_b_ap = bass.AP(tensor=s_slice.tensor, offset=s_slice.offset,
                                 ap=[[0, 128]] + list(s_slice.ap))
                nc.gpsimd.dma_start(out=s_bc, in_=s_b_ap)

                # ages rows, pre-divided by s (i.e. * rms)
                a12c = spool.tile([2, TC], f32, tag="a12c", bufs=1,
                                  name=f"a12c_{c}")
                nc.gpsimd.dma_start(out=a12c, in_=a12_d[:, tok0:tok0 + TC])
                rms2 = spool.tile([2, TC], f32, tag="rms2", bufs=1,
                                  name=f"rms2_{c}")
                nc.vector.reciprocal(rms2, s_bc[0:2, :])
                a12s = spool.tile([2, TC], bf16, tag="a12s", bufs=1,
                                  name=f"a12s_{c}")
                nc.vector.tensor_mul(a12s, a12c, rms2)

                # ---- load xT chunk (hid-major) --------------------------
                xTt = []
                for k in range(KH):
                    xk = xpool.tile([128, TC], bf16, tag="xT",
                                    name=f"xT_{c}_{k}")
                    if k == KH - 1:
                        nc.sync.dma_start(
                            out=xk[0:126, :],
                            in_=xT_d[k * 128:k * 128 + 126, tok0:tok0 + TC])
                        nc.gpsimd.dma_start(out=xk[126:128, :], in_=a12s)
                    else:
                        nc.sync.dma_start(
                            out=xk,
                            in_=xT_d[k * 128:(k + 1) * 128, tok0:tok0 + TC])
                    xTt.append(xk)

                # ---- v projection (token-major) -------------------------
                for tsub in range(NT):
                    pv = ps_mm.tile([128, HPC * 128], f32, tag="a",
                                    name=f"pv_{c}_{tsub}")
                    for k in range(KH):
                        nc.tensor.matmul(
                            pv, lhsT=xTt[k][:, tsub * 128:(tsub + 1) * 128],
                            rhs=wv_sb[:, k, :],
                            start=(k == 0), stop=(k == KH - 1))
                    j = tok0 // 128 + tsub
                    for h in range(HPC):
                        nc.vector.tensor_scalar_mul(
                            v_sb[:, h, j, 0:128], pv[:, h * 128:(h + 1) * 128],
                            s_tiles[tsub])
                        nc.vector.memset(v_sb[:, h, j, 128:129], 1.0)

                # ---- fused W_in matmul (transposed out) -----------------
                # m order: g1_0, g2_0, ..., g1_7, g2_7, qA, qB, kA, kB
                silu_t = {}
                g2_t = {}
                qk_raw = {}
                for m in range(2 * NFF + 2 * HPC):
                    wmt = wmp.tile([128, KH, 128], bf16, tag="wm",
                                   name=f"wm_{c}_{m}")
                    nc.scalar.dma_start(out=wmt, in_=win_d[m])
                    for n in range(NW):
                        nsl = slice(n * WN, (n + 1) * WN)
                        pm = ps_mm.tile([128, WN], f32, tag="a",
                                        name=f"pm_{c}_{m}_{n}")
                        for k in range(KH):
                            nc.tensor.matmul(pm, lhsT=wmt[:, k, :],
                                             rhs=xTt[k][:, nsl],
                                             start=(k == 0),
                                             stop=(k == KH - 1))
                        if m < 2 * NFF and m % 2 == 0:      # g1
                            p = m // 2
                            t1 = evictp.tile([128, TC], bf16, tag="g1",
                                             name=f"g1_{c}_{p}")
                            if p not in silu_t:
                                silu_t[p] = (t1, evictp.tile(
                                    [128, TC], bf16, tag="silu",
                                    name=f"silu_{c}_{p}"))
                            g1t, st = silu_t[p]
                            nc.vector.tensor_mul(g1t[:, nsl], pm, s_bc[:, nsl])
                            nc.scalar.activation(st[:, nsl], g1t[:, nsl],
                                                 AF.Silu)
                        elif m < 2 * NFF:                    # g2
                            p = m // 2
                            if p not in g2_t:
                                g2_t[p] = evictp.tile([128, TC], bf16,
                                                      tag="g2",
                                                      name=f"g2_{c}_{p}")
                            g2t = g2_t[p]
                            nc.vector.tensor_mul(g2t[:, nsl], pm, s_bc[:, nsl])
                        else:                                # q or k
                            qi = m - 2 * NFF
                            if qi not in qk_raw:
                                qk_raw[qi] = qkp.tile([128, TC], bf16,
                                                      tag="qkraw",
                                                      name=f"qkraw_{c}_{qi}")
                            nc.vector.tensor_mul(qk_raw[qi][:, nsl], pm,
                                                 s_bc[:, nsl])

                # swiglu: ff = silu(g1) * g2  -> combT tiles 0..NFF-1
                combT = combp.tile([128, NCOMB, TC], bf16, tag="comb",
                                   name=f"combT_{c}")
                for p in range(NFF):
                    nc.vector.tensor_mul(combT[:, p, :], silu_t[p][1],
                                         g2_t[p])

                # ---- rope ----------------------------------------------
                cos_sb = qkp.tile([128, TC], bf16, tag="cos", bufs=2,
                                  name=f"cos_{c}")
                nc.gpsimd.dma_start(out=cos_sb, in_=cos_d[:, tok0:tok0 + TC])
                sin_sb = qkp.tile([128, TC], bf16, tag="sin", bufs=2,
                                  name=f"sin_{c}")
                nc.gpsimd.dma_start(out=sin_sb, in_=sin_d[:, tok0:tok0 + TC])

                qT = qkp.tile([128, HPC, TC], bf16, tag="qT", bufs=2,
                              name=f"qT_{c}")
                # (qi, destination slice): q -> qT chunk, k -> resident kT
                rope_jobs = [(h, qT[:, h, :]) for h in range(HPC)]
                rope_jobs += [(HPC + h, kT[:, h, tok0:tok0 + TC])
                              for h in range(HPC)]
                for qi, dst in rope_jobs:
                    src = qk_raw[qi]
                    for n in range(NW):
                        nsl = slice(n * WN, (n + 1) * WN)
                        psw = ps_misc.tile([128, WN], f32, tag="a",
                                           name=f"psw_{c}_{qi}_{n}")
                        nc.tensor.matmul(psw, lhsT=swap_sb, rhs=src[:, nsl],
                                         start=True, stop=True)
                        rt1 = qkp.tile([128, WN], bf16, tag="rt1", bufs=2,
                                       name=f"rt1_{c}_{qi}_{n}")
                        nc.vector.tensor_mul(rt1, psw, sin_sb[:, nsl])
                        rt2 = qkp.tile([128, WN], bf16, tag="rt2", bufs=2,
                                       name=f"rt2_{c}_{qi}_{n}")
                        nc.vector.tensor_mul(rt2, src[:, nsl], cos_sb[:, nsl])
                        nc.vector.tensor_add(dst[:, nsl], rt1, rt2)

                # ---- causal attention ----------------------------------
                for qc in range(NQ):
                    q0 = tok0 + qc * QC
                    kmax = (q0 + QC) // 128
                    for h in range(HPC):
                        pa = [ps_attn.tile([128, 129], f32, tag="attn",
                                           name=f"pa_{c}_{qc}_{h}_{i}")
                              for i in range(NB)]
                        for j in range(kmax):
                            psc = ps_misc.tile([128, QC], f32, tag="a",
                                               name=f"psc_{c}_{qc}_{h}_{j}")
                            nc.tensor.matmul(
                                psc, lhsT=kT[:, h, j * 128:(j + 1) * 128],
                                rhs=qT[:, h, qc * QC:(qc + 1) * QC],
                                start=True, stop=True)
                            pT = ppool.tile([128, QC], bf16, tag="p",
                                            name=f"pT_{c}_{qc}_{h}_{j}")
                            nc.scalar.activation(pT, psc, AF.Exp, scale=SCALE)
                            D = j * 128 - q0
                            if D >= 0:
                                nc.vector.tensor_mul(
                                    pT, pT, mask_sb[:, 384 - D:384 - D + QC])
                            for b in range(NB):
                                nc.tensor.matmul(
                                    pa[b],
                                    lhsT=pT[:, b * 128:(b + 1) * 128],
                                    rhs=v_sb[:, h, j, :],
                                    start=(j == 0), stop=(j == kmax - 1))
                        # normalize + transpose into combT
                        for b in range(NB):
                            li = attnp.tile([128, 1], f32, tag="l",
                                            name=f"l_{c}_{qc}_{h}_{b}")
                            nc.vector.reciprocal(li, pa[b][:, 128:129])
                            at = attnp.tile([128, 128], bf16, tag="at",
                                            name=f"at_{c}_{qc}_{h}_{b}")
                            nc.vector.tensor_scalar_mul(
                                at, pa[b][:, 0:128], li)
                            ptr = ps_misc.tile([128, 128], bf16, tag="a",
                                               name=f"ptr_{c}_{qc}_{h}_{b}")
                            nc.tensor.transpose(ptr, at, ident_sb)
                            col0 = qc * QC + b * 128
                            nc.scalar.copy(
                                combT[:, NFF + h, col0:col0 + 128], ptr)

                # ---- output projection (token-major) --------------------
                for oc in range(NO):
                    wot = wop.tile([128, NCOMB, 512], bf16, tag="wo", bufs=2,
                                   name=f"wo_{c}_{oc}")
                    nc.scalar.dma_start(out=wot, in_=wo_d[oc])
                    for tsub in range(NT):
                        po = ps_out.tile([128, 512], f32, tag="out",
                                         name=f"po_{c}_{oc}_{tsub}")
                        for kc in range(NCOMB):
                            nc.tensor.matmul(
                                po,
                                lhsT=combT[:, kc,
                                           tsub * 128:(tsub + 1) * 128],
                                rhs=wot[:, kc, :],
                                start=(kc == 0), stop=(kc == NCOMB - 1))
                        ost = outp.tile([128, 512], f32, tag="ost",
                                        name=f"ost_{c}_{oc}_{tsub}")
                        nc.vector.tensor_copy(ost, po)
                        r0 = tok0 + tsub * 128
                        nc.sync.dma_start(
                            out=acc[r0:r0 + 128, oc * 512:(oc + 1) * 512],
                            in_=ost)

                # ---- reduce-scatter this chunk's partial output ---------
                nc.gpsimd.collective_compute(
                    "ReduceScatter",
                    mybir.AluOpType.add,
                    replica_groups=[list(range(NCORES))],
                    ins=[acc[tok0:tok0 + TC, :]],
                    outs=[rs_out[c]],
                )
                nc.sync.dma_start(out=out_d[c], in_=rs_out[c])



    nc.compile()
    return nc


def _prep_in_maps(x, normed_ages, sin, cos, norm_w, W_in, W_out):
    """Shard + preprocess inputs into per-core in_maps (numpy only)."""
    T = x.shape[0]
    xT_bf = np.ascontiguousarray(x.T).astype(BF16)
    xtok_bf = x.astype(BF16)
    cos_t = np.ascontiguousarray(cos.reshape(T, HD).T).astype(BF16)
    sin_t = np.ascontiguousarray(sin.reshape(T, HD).T).astype(BF16)
    a12 = np.stack([normed_ages, normed_ages * normed_ages]).astype(np.float32)

    sw = np.zeros((128, 128), np.float32)
    idx = np.arange(0, 128, 2)
    sw[idx + 1, idx] = -1.0   # lhsT[2i+1, 2i] = -1
    sw[idx, idx + 1] = 1.0    # lhsT[2i, 2i+1] = +1
    swapmat = sw.astype(BF16)

    maskbase = (np.arange(896)[None, :] - 384 >=
                np.arange(128)[:, None]).astype(BF16)
    identity = np.eye(128, dtype=np.float32).astype(BF16)

    # norm_w folded into W_in except the last two hid columns (the
    # normed_ages overwrite bypasses the norm weight).
    def fold(wrows):
        w = wrows * norm_w[None, :]
        w[:, HID - 2:] = wrows[:, HID - 2:]
        return w

    q_base = 2 * INTER
    k_base = 2 * INTER + HID
    v_base = 2 * INTER + 2 * HID

    in_maps = []
    for core in range(NCORES):
        f0 = FPC * core
        h0 = HPC * core
        rows = []
        for p in range(NFF):
            rows.append(W_in[f0 + p * 128: f0 + (p + 1) * 128])           # g1_p
            rows.append(W_in[INTER + f0 + p * 128:
                             INTER + f0 + (p + 1) * 128])                 # g2_p
        for h in range(HPC):
            rows.append(W_in[q_base + (h0 + h) * HD:
                             q_base + (h0 + h + 1) * HD])                 # q
        for h in range(HPC):
            rows.append(W_in[k_base + (h0 + h) * HD:
                             k_base + (h0 + h + 1) * HD])                 # k
        w_used = fold(np.concatenate(rows, axis=0))                       # [2560, HID]
        nm = 2 * NFF + 2 * HPC
        # [m, p(hid-in-tile), k, j(row-in-tile)] so each partition is linear
        w_in_t = np.ascontiguousarray(
            w_used.reshape(nm, 128, KH, 128).transpose(0, 3, 2, 1)
        ).astype(BF16)

        wv = fold(W_in[v_base + h0 * HD: v_base + (h0 + HPC) * HD])       # [256, HID]
        w_v_t = np.ascontiguousarray(
            wv.reshape(HPC * 128, KH, 128).transpose(2, 1, 0)).astype(BF16)

        # W_out columns in comb order: ff block, then attn heads
        cols = list(range(HID + f0, HID + f0 + FPC))
        for h in range(HPC):
            cols += list(range((h0 + h) * HD, (h0 + h + 1) * HD))
        w_o_loc_t = np.ascontiguousarray(W_out[:, cols].T)                # [1280, HID]
        # [oc, p(c-in-tile), kc, ow] so each partition is linear per oc
        w_out_t = np.ascontiguousarray(
            w_o_loc_t.reshape(NCOMB, 128, HID // 512, 512)
            .transpose(2, 1, 0, 3)).astype(BF16)

        in_maps.append({
            "xt": xT_bf, "xtok": xtok_bf,
            "w_in_t": w_in_t, "w_v_t": w_v_t, "w_out_t": w_out_t,
            "cos_t": cos_t, "sin_t": sin_t, "a12": a12,
            "swapmat": swapmat, "maskbase": maskbase, "identity": identity,
        })
    return in_maps


_NC_CACHE = {}


def get_nc(T=T_FULL, TC=512):
    key = (T, TC)
    if key not in _NC_CACHE:
        _NC_CACHE[key] = _build_nc(T, TC)
    return _NC_CACHE[key]


def run(x, normed_ages, sin, cos, norm_w, W_in, W_out, T=T_FULL, TC=512,
        trace=False):
    from concourse.bass_utils import run_bass_kernel_spmd
    nc = get_nc(T, TC)
    in_maps = _prep_in_maps(x, normed_ages, sin, cos, norm_w, W_in, W_out)
    res = run_bass_kernel_spmd(nc, in_maps, list(range(NCORES)), trace=trace)
    # results[i]["out"][c] holds reduced rows [c*TC + i*(TC/8) : +TC/8]
    nchunk = T // TC
    seg = TC // NCORES
    out = np.empty((T, HID), np.float32)
    for i in range(NCORES):
        oi = np.asarray(res.results[i]["out"], np.float32)
        for c in range(nchunk):
            r0 = c * TC + i * seg
            out[r0:r0 + seg] = oi[c]
    return out, res


def kernel(x, normed_ages, sin, cos, norm_w, W_in, W_out):
    out, _ = run(x, normed_ages, sin, cos, norm_w, W_in, W_out)
    return out


# revision 10
# speedup vs baseline: 1.1947x; 1.0114x over previous
"""Trainium2 Bass kernel for a dense transformer decoder layer.

Tensor-parallel across 8 NeuronCores:
  - heads: 2 per core (of 16), ff channels: 1024 per core (of 8192)
  - W_in rows / W_out cols sharded accordingly; ReduceScatter(add) of the
    partial outputs at the end; host concatenates the 8 shards.

Per-core dataflow (token chunks of TC):
  stats (token-major x) -> s = rsqrt(mean(x^2)+eps) -> DRAM round-trip for a
  partition broadcast; RMSNorm is folded into the matmul eviction
  (t = s * (W @ x~)) with norm_w folded into W on the host and the
  normed_ages overwrite handled by patching the last two hid rows of x~ with
  a12 * rms.  W_in matmul produces q/k transposed ([hd, tok]; rope applied
  via a pairwise-swap matmul on the PE + two multiplies), v in token-major
  form ([tok, hd]) via a second matmul orientation, and the swiglu branch.
  Causal attention runs with k-token-major score tiles, exp without
  max-subtraction (scores are O(5) here so fp32/bf16 exp is safe), a
  multiplicative causal mask on diagonal blocks, and the softmax denominator
  picked up for free through an appended ones-column on v.  The combined
  [ff|attn] activations feed the W_out matmul in token-major form, written to
  DRAM and reduce-scattered.
"""

import os
import sys

for _p in ("/opt/trn_rl_repo", "/opt/pypackages"):
    if _p not in sys.path:
        sys.path.insert(0, _p)

import numpy as np
import ml_dtypes

BF16 = ml_dtypes.bfloat16

# Model dims (fixed by the problem)
T_FULL = 4096
HID = 2048
NH = 16
HD = 128
INTER = 8192
EPS = 1e-6
SCALE = 1.0 / float(np.sqrt(np.float32(HD)))

NCORES = 8
HPC = NH // NCORES          # heads per core = 2
FPC = INTER // NCORES       # ff channels per core = 1024
NFF = FPC // 128            # ff m-tiles per core (per g1/g2) = 8
NCOMB = NFF + HPC           # comb k-tiles: ff + one per head = 10
KH = HID // 128             # hid k-tiles = 16


def _build_nc(T, TC):
    import concourse.bass as bass
    import concourse.tile as tile
    from concourse import bacc, mybir

    f32 = mybir.dt.float32
    bf16 = mybir.dt.bfloat16
    AF = mybir.ActivationFunctionType
    X = mybir.AxisListType.X

    NCHUNK = T // TC
    QC = min(512, TC)            # attention q-chunk width
    NQ = TC // QC                # q-chunks per token chunk
    NB = QC // 128               # q-subblocks per q-chunk
    NW = max(TC // 512, 1)       # 512-wide n-chunks per token chunk
    WN = min(512, TC)            # n-chunk width for W_in matmul
    NT = TC // 128               # token subtiles per chunk
    NO = HID // 512              # output col chunks = 4
    JT = T // 128                # total k-blocks (tok tiles) over full T

    nc = bacc.Bacc("TRN2", target_bir_lowering=False, debug=False,
                   num_devices=NCORES)

    # ---- DRAM parameters -------------------------------------------------
    xT_d = nc.dram_tensor("xt", [HID, T], bf16, kind="ExternalInput").ap()
    xtok_d = nc.dram_tensor("xtok", [T, HID], bf16, kind="ExternalInput").ap()
    win_d = nc.dram_tensor("w_in_t", [2 * NFF + 2 * HPC, 128, KH, 128], bf16,
                           kind="ExternalInput").ap()
    wv_d = nc.dram_tensor("w_v_t", [128, KH, HPC * 128], bf16,
                          kind="ExternalInput").ap()
    wo_d = nc.dram_tensor("w_out_t", [NO, 128, NCOMB, 512], bf16,
                          kind="ExternalInput").ap()
    cos_d = nc.dram_tensor("cos_t", [HD, T], bf16, kind="ExternalInput").ap()
    sin_d = nc.dram_tensor("sin_t", [HD, T], bf16, kind="ExternalInput").ap()
    a12_d = nc.dram_tensor("a12", [2, T], f32, kind="ExternalInput").ap()
    swap_d = nc.dram_tensor("swapmat", [128, 128], bf16,
                            kind="ExternalInput").ap()
    mask_d = nc.dram_tensor("maskbase", [128, 896], bf16,
                            kind="ExternalInput").ap()
    ident_d = nc.dram_tensor("identity", [128, 128], bf16,
                             kind="ExternalInput").ap()
    out_d = nc.dram_tensor("out", [NCHUNK, TC // NCORES, HID], f32,
                           kind="ExternalOutput").ap()

    from contextlib import ExitStack

    with tile.TileContext(nc) as tc:
        with ExitStack() as ctx:
            const = ctx.enter_context(tc.tile_pool(name="const", bufs=1))
            kv = ctx.enter_context(tc.tile_pool(name="kv", bufs=1))
            dram = ctx.enter_context(
                tc.tile_pool(name="dram", bufs=1, space="DRAM"))
            xpool = ctx.enter_context(tc.tile_pool(name="xpool", bufs=KH + 6))
            xtokp = ctx.enter_context(tc.tile_pool(name="xtokp", bufs=3))
            statp = ctx.enter_context(tc.tile_pool(name="statp", bufs=3))
            spool = ctx.enter_context(tc.tile_pool(name="spool", bufs=2))
            stiles = ctx.enter_context(
                tc.tile_pool(name="stiles", bufs=2 * NT + 2))
            wmp = ctx.enter_context(tc.tile_pool(name="wmp", bufs=6))
            evictp = ctx.enter_context(tc.tile_pool(name="evictp", bufs=2))
            qkp = ctx.enter_context(tc.tile_pool(name="qkp", bufs=4))
            combp = ctx.enter_context(tc.tile_pool(name="combp", bufs=1))
            ppool = ctx.enter_context(tc.tile_pool(name="ppool", bufs=4))
            attnp = ctx.enter_context(tc.tile_pool(name="attnp", bufs=4))
            wop = ctx.enter_context(tc.tile_pool(name="wop", bufs=12))
            outp = ctx.enter_context(tc.tile_pool(name="outp", bufs=4))
            ps_mm = ctx.enter_context(
                tc.tile_pool(name="ps_mm", bufs=2, space="PSUM"))
            ps_misc = ps_mm
            ps_attn = ctx.enter_context(
                tc.tile_pool(name="ps_attn", bufs=4, space="PSUM"))
            ps_out = ctx.enter_context(
                tc.tile_pool(name="ps_out", bufs=2, space="PSUM"))
            # ---- constants ----------------------------------------------
            swap_sb = const.tile([128, 128], bf16, name="swap_sb")
            nc.sync.dma_start(out=swap_sb, in_=swap_d)
            mask_sb = const.tile([128, 896], bf16, name="mask_sb")
            nc.sync.dma_start(out=mask_sb, in_=mask_d)
            ident_sb = const.tile([128, 128], bf16, name="ident_sb")
            nc.sync.dma_start(out=ident_sb, in_=ident_d)
            eps_sb = const.tile([128, 1], f32, name="eps_sb")
            nc.vector.memset(eps_sb, EPS)
            # v-projection weights, resident: [128 hid-part, KH, HPC*128]
            wv_sb = const.tile([128, KH, HPC * 128], bf16, name="wv_sb")
            nc.sync.dma_start(out=wv_sb, in_=wv_d)

            # persistent K / V (token history)
            kT = kv.tile([128, HPC, T], bf16, name="kT")
            v_sb = kv.tile([128, HPC, JT, 129], bf16, name="v_sb")

            # DRAM scratch (acc is per-chunk; see chunk loop)
            rs_out = dram.tile([NCHUNK, TC // NCORES, HID], f32,
                               name="rs_out")

            for c in range(NCHUNK):
                tok0 = c * TC

                # ---- stats: s = 1/sqrt(mean(x^2)+eps), per token --------
                s_dram = dram.tile([TC], f32, tag="s_dram", bufs=2,
                                   name=f"s_dram_{c}")
                acc_c = dram.tile([TC, HID], f32, tag="acc", bufs=3,
                                  name=f"acc_{c}")
                s_tiles = []
                for tt in range(NT):
                    r0 = tok0 + tt * 128
                    xt = xtokp.tile([128, HID], bf16, tag="xtok",
                                    name=f"xt_{c}_{tt}")
                    nc.gpsimd.dma_start(out=xt, in_=xtok_d[r0:r0 + 128, :])
                    xsq = statp.tile([128, HID], bf16, tag="xsq", bufs=2,
                                     name=f"xsq_{c}_{tt}")
                    nc.vector.tensor_mul(xsq, xt, xt)
                    ssum = statp.tile([128, 1], f32, tag="ssum",
                                      name=f"ssum_{c}_{tt}")
                    nc.vector.reduce_sum(ssum, xsq, axis=X)
                    nc.scalar.activation(ssum, ssum, AF.Sqrt, bias=eps_sb,
                                         scale=1.0 / HID)
                    s_t = stiles.tile([128, 1], f32, tag="s",
                                      name=f"s_{c}_{tt}")
                    nc.vector.reciprocal(s_t, ssum)
                    s_tiles.append(s_t)
                    nc.gpsimd.dma_start(out=s_dram[tt * 128:(tt + 1) * 128], in_=s_t)

                # broadcast s over partitions via DRAM round-trip
                s_bc = spool.tile([128, TC], f32, tag="sbc",
                                  name=f"sbc_{c}")
                s_slice = s_dram[:]
                s_b_ap = bass.AP(tensor=s_slice.tensor, offset=s_slice.offset,
                                 ap=[[0, 128]] + list(s_slice.ap))
                nc.gpsimd.dma_start(out=s_bc, in_=s_b_ap)

                # ages rows, pre-divided by s (i.e. * rms)
                a12c = spool.tile([2, TC], f32, tag="a12c", bufs=1,
                                  name=f"a12c_{c}")
                nc.gpsimd.dma_start(out=a12c, in_=a12_d[:, tok0:tok0 + TC])
                rms2 = spool.tile([2, TC], f32, tag="rms2", bufs=1,
                                  name=f"rms2_{c}")
                nc.vector.reciprocal(rms2, s_bc[0:2, :])
                a12s = spool.tile([2, TC], bf16, tag="a12s", bufs=1,
                                  name=f"a12s_{c}")
                nc.vector.tensor_mul(a12s, a12c, rms2)

                # ---- load xT chunk (hid-major) --------------------------
                xTt = []
                for k in range(KH):
                    xk = xpool.tile([128, TC], bf16, tag="xT",
                                    name=f"xT_{c}_{k}")
                    if k == KH - 1:
                        nc.sync.dma_start(
                            out=xk[0:126, :],
                            in_=xT_d[k * 128:k * 128 + 126, tok0:tok0 + TC])
                        nc.gpsimd.dma_start(out=xk[126:128, :], in_=a12s)
                    else:
                        nc.sync.dma_start(
                            out=xk,
                            in_=xT_d[k * 128:(k + 1) * 128, tok0:tok0 + TC])
                    xTt.append(xk)

                # ---- v projection (token-major) -------------------------
                for tsub in range(NT):
                    pv = ps_mm.tile([128, HPC * 128], f32, tag="a",
                                    name=f"pv_{c}_{tsub}")
                    for k in range(KH):
                        nc.tensor.matmul(
                            pv, lhsT=xTt[k][:, tsub * 128:(tsub + 1) * 128],
                            rhs=wv_sb[:, k, :],
                            start=(k == 0), stop=(k == KH - 1))
                    j = tok0 // 128 + tsub
                    for h in range(HPC):
                        nc.vector.tensor_scalar_mul(
                            v_sb[:, h, j, 0:128], pv[:, h * 128:(h + 1) * 128],
                            s_tiles[tsub])
                        nc.vector.memset(v_sb[:, h, j, 128:129], 1.0)

                # ---- fused W_in matmul (transposed out) -----------------
                # m order: g1_0, g2_0, ..., g1_7, g2_7, qA, qB, kA, kB
                silu_t = {}
                g2_t = {}
                qk_raw = {}
                for m in range(2 * NFF + 2 * HPC):
                    wmt = wmp.tile([128, KH, 128], bf16, tag="wm",
                                   name=f"wm_{c}_{m}")
                    nc.scalar.dma_start(out=wmt, in_=win_d[m])
                    for n in range(NW):
                        nsl = slice(n * WN, (n + 1) * WN)
                        pm = ps_mm.tile([128, WN], f32, tag="a",
                                        name=f"pm_{c}_{m}_{n}")
                        for k in range(KH):
                            nc.tensor.matmul(pm, lhsT=wmt[:, k, :],
                                             rhs=xTt[k][:, nsl],
                                             start=(k == 0),
                                             stop=(k == KH - 1))
                        if m < 2 * NFF and m % 2 == 0:      # g1
                            p = m // 2
                            t1 = evictp.tile([128, TC], bf16, tag="g1",
                                             name=f"g1_{c}_{p}")
                            if p not in silu_t:
                                silu_t[p] = (t1, evictp.tile(
                                    [128, TC], bf16, tag="silu",
                                    name=f"silu_{c}_{p}"))
                            g1t, st = silu_t[p]
                            nc.vector.tensor_mul(g1t[:, nsl], pm, s_bc[:, nsl])
                            nc.scalar.activation(st[:, nsl], g1t[:, nsl],
                                                 AF.Silu)
                        elif m < 2 * NFF:                    # g2
                            p = m // 2
                            if p not in g2_t:
                                g2_t[p] = evictp.tile([128, TC], bf16,
                                                      tag="g2",
                                                      name=f"g2_{c}_{p}")
                            g2t = g2_t[p]
                            nc.vector.tensor_mul(g2t[:, nsl], pm, s_bc[:, nsl])
                        else:                                # q or k
                            qi = m - 2 * NFF
                            if qi not in qk_raw:
                                qk_raw[qi] = qkp.tile([128, TC], bf16,
                                                      tag="qkraw",
                                                      name=f"qkraw_{c}_{qi}")
                            nc.vector.tensor_mul(qk_raw[qi][:, nsl], pm,
                                                 s_bc[:, nsl])

                # swiglu: ff = silu(g1) * g2  -> combT tiles 0..NFF-1
                combT = combp.tile([128, NCOMB, TC], bf16, tag="comb",
                                   name=f"combT_{c}")
                for p in range(NFF):
                    nc.vector.tensor_mul(combT[:, p, :], silu_t[p][1],
                                         g2_t[p])

                # ---- rope ----------------------------------------------
                cos_sb = qkp.tile([128, TC], bf16, tag="cos", bufs=2,
                                  name=f"cos_{c}")
                nc.gpsimd.dma_start(out=cos_sb, in_=cos_d[:, tok0:tok0 + TC])
                sin_sb = qkp.tile([128, TC], bf16, tag="sin", bufs=2,
                                  name=f"sin_{c}")
                nc.gpsimd.dma_start(out=sin_sb, in_=sin_d[:, tok0:tok0 + TC])

                qT = qkp.tile([128, HPC, TC], bf16, tag="qT", bufs=2,
                              name=f"qT_{c}")
                # (qi, destination slice): q -> qT chunk, k -> resident kT
                rope_jobs = [(h, qT[:, h, :]) for h in range(HPC)]
                rope_jobs += [(HPC + h, kT[:, h, tok0:tok0 + TC])
                              for h in range(HPC)]
                for qi, dst in rope_jobs:
                    src = qk_raw[qi]
                    for n in range(NW):
                        nsl = slice(n * WN, (n + 1) * WN)
                        psw = ps_misc.tile([128, WN], f32, tag="a",
                                           name=f"psw_{c}_{qi}_{n}")
                        nc.tensor.matmul(psw, lhsT=swap_sb, rhs=src[:, nsl],
                                         start=True, stop=True)
                        rt1 = qkp.tile([128, WN], bf16, tag="rt1", bufs=2,
                                       name=f"rt1_{c}_{qi}_{n}")
                        nc.vector.tensor_mul(rt1, psw, sin_sb[:, nsl])
                        rt2 = qkp.tile([128, WN], bf16, tag="rt2", bufs=2,
                                       name=f"rt2_{c}_{qi}_{n}")
                        nc.vector.tensor_mul(rt2, src[:, nsl], cos_sb[:, nsl])
                        nc.vector.tensor_add(dst[:, nsl], rt1, rt2)

                # ---- causal attention ----------------------------------
                for qc in range(NQ):
                    q0 = tok0 + qc * QC
                    kmax = (q0 + QC) // 128
                    for h in range(HPC):
                        pa = [ps_attn.tile([128, 129], f32, tag="attn",
                                           name=f"pa_{c}_{qc}_{h}_{i}")
                              for i in range(NB)]
                        for j in range(kmax):
                            psc = ps_misc.tile([128, QC], f32, tag="a",
                                               name=f"psc_{c}_{qc}_{h}_{j}")
                            nc.tensor.matmul(
                                psc, lhsT=kT[:, h, j * 128:(j + 1) * 128],
                                rhs=qT[:, h, qc * QC:(qc + 1) * QC],
                                start=True, stop=True)
                            pT = ppool.tile([128, QC], bf16, tag="p",
                                            name=f"pT_{c}_{qc}_{h}_{j}")
                            nc.scalar.activation(pT, psc, AF.Exp, scale=SCALE)
                            D = j * 128 - q0
                            if D >= 0:
                                nc.vector.tensor_mul(
                                    pT, pT, mask_sb[:, 384 - D:384 - D + QC])
                            for b in range(NB):
                                nc.tensor.matmul(
                                    pa[b],
                                    lhsT=pT[:, b * 128:(b + 1) * 128],
                                    rhs=v_sb[:, h, j, :],
                                    start=(j == 0), stop=(j == kmax - 1))
                        # normalize + transpose into combT
                        for b in range(NB):
                            li = attnp.tile([128, 1], f32, tag="l",
                                            name=f"l_{c}_{qc}_{h}_{b}")
                            nc.vector.reciprocal(li, pa[b][:, 128:129])
                            at = attnp.tile([128, 128], bf16, tag="at",
                                            name=f"at_{c}_{qc}_{h}_{b}")
                            nc.vector.tensor_scalar_mul(
                                at, pa[b][:, 0:128], li)
                            ptr = ps_misc.tile([128, 128], bf16, tag="a",
                                               name=f"ptr_{c}_{qc}_{h}_{b}")
                            nc.tensor.transpose(ptr, at, ident_sb)
                            col0 = qc * QC + b * 128
                            nc.scalar.copy(
                                combT[:, NFF + h, col0:col0 + 128], ptr)

                # ---- output projection (token-major) --------------------
                for oc in range(NO):
                    wot = wop.tile([128, NCOMB, 512], bf16, tag="wo", bufs=2,
                                   name=f"wo_{c}_{oc}")
                    nc.scalar.dma_start(out=wot, in_=wo_d[oc])
                    for tsub in range(NT):
                        po = ps_out.tile([128, 512], f32, tag="out",
                                         name=f"po_{c}_{oc}_{tsub}")
                        for kc in range(NCOMB):
                            nc.tensor.matmul(
                                po,
                                lhsT=combT[:, kc,
                                           tsub * 128:(tsub + 1) * 128],
                                rhs=wot[:, kc, :],
                                start=(kc == 0), stop=(kc == NCOMB - 1))
                        ost = outp.tile([128, 512], f32, tag="ost",
                                        name=f"ost_{c}_{oc}_{tsub}")
                        nc.vector.tensor_copy(ost, po)
                        r0 = tsub * 128
                        nc.sync.dma_start(
                            out=acc_c[r0:r0 + 128, oc * 512:(oc + 1) * 512],
                            in_=ost)

                # ---- reduce-scatter this chunk's partial output ---------
                nc.gpsimd.collective_compute(
                    "ReduceScatter",
                    mybir.AluOpType.add,
                    replica_groups=[list(range(NCORES))],
                    ins=[acc_c[:, :]],
                    outs=[rs_out[c]],
                )
                nc.sync.dma_start(out=out_d[c], in_=rs_out[c])



    nc.compile()
    return nc


def _prep_in_maps(x, normed_ages, sin, cos, norm_w, W_in, W_out):
    """Shard + preprocess inputs into per-core in_maps (numpy only)."""
    T = x.shape[0]
    xT_bf = np.ascontiguousarray(x.T).astype(BF16)
    xtok_bf = x.astype(BF16)
    cos_t = np.ascontiguousarray(cos.reshape(T, HD).T).astype(BF16)
    sin_t = np.ascontiguousarray(sin.reshape(T, HD).T).astype(BF16)
    a12 = np.stack([normed_ages, normed_ages * normed_ages]).astype(np.float32)

    sw = np.zeros((128, 128), np.float32)
    idx = np.arange(0, 128, 2)
    sw[idx + 1, idx] = -1.0   # lhsT[2i+1, 2i] = -1
    sw[idx, idx + 1] = 1.0    # lhsT[2i, 2i+1] = +1
    swapmat = sw.astype(BF16)

    maskbase = (np.arange(896)[None, :] - 384 >=
                np.arange(128)[:, None]).astype(BF16)
    identity = np.eye(128, dtype=np.float32).astype(BF16)

    # norm_w folded into W_in except the last two hid columns (the
    # normed_ages overwrite bypasses the norm weight).
    def fold(wrows):
        w = wrows * norm_w[None, :]
        w[:, HID - 2:] = wrows[:, HID - 2:]
        return w

    q_base = 2 * INTER
    k_base = 2 * INTER + HID
    v_base = 2 * INTER + 2 * HID

    in_maps = []
    for core in range(NCORES):
        f0 = FPC * core
        h0 = HPC * core
        rows = []
        for p in range(NFF):
            rows.append(W_in[f0 + p * 128: f0 + (p + 1) * 128])           # g1_p
            rows.append(W_in[INTER + f0 + p * 128:
                             INTER + f0 + (p + 1) * 128])                 # g2_p
        for h in range(HPC):
            rows.append(W_in[q_base + (h0 + h) * HD:
                             q_base + (h0 + h + 1) * HD])                 # q
        for h in range(HPC):
            rows.append(W_in[k_base + (h0 + h) * HD:
                             k_base + (h0 + h + 1) * HD])                 # k
        w_used = fold(np.concatenate(rows, axis=0))                       # [2560, HID]
        nm = 2 * NFF + 2 * HPC
        # [m, p(hid-in-tile), k, j(row-in-tile)] so each partition is linear
        w_in_t = np.ascontiguousarray(
            w_used.reshape(nm, 128, KH, 128).transpose(0, 3, 2, 1)
        ).astype(BF16)

        wv = fold(W_in[v_base + h0 * HD: v_base + (h0 + HPC) * HD])       # [256, HID]
        w_v_t = np.ascontiguousarray(
            wv.reshape(HPC * 128, KH, 128).transpose(2, 1, 0)).astype(BF16)

        # W_out columns in comb order: ff block, then attn heads
        cols = list(range(HID + f0, HID + f0 + FPC))
        for h in range(HPC):
            cols += list(range((h0 + h) * HD, (h0 + h + 1) * HD))
        w_o_loc_t = np.ascontiguousarray(W_out[:, cols].T)                # [1280, HID]
        # [oc, p(c-in-tile), kc, ow] so each partition is linear per oc
        w_out_t = np.ascontiguousarray(
            w_o_loc_t.reshape(NCOMB, 128, HID // 512, 512)
            .transpose(2, 1, 0, 3)).astype(BF16)

        in_maps.append({
            "xt": xT_bf, "xtok": xtok_bf,
            "w_in_t": w_in_t, "w_v_t": w_v_t, "w_out_t": w_out_t,
            "cos_t": cos_t, "sin_t": sin_t, "a12": a12,
            "swapmat": swapmat, "maskbase": maskbase, "identity": identity,
        })
    return in_maps


_NC_CACHE = {}


def get_nc(T=T_FULL, TC=512):
    key = (T, TC)
    if key not in _NC_CACHE:
        _NC_CACHE[key] = _build_nc(T, TC)
    return _NC_CACHE[key]


def run(x, normed_ages, sin, cos, norm_w, W_in, W_out, T=T_FULL, TC=512,
        trace=False):
    from concourse.bass_utils import run_bass_kernel_spmd
    nc = get_nc(T, TC)
    in_maps = _prep_in_maps(x, normed_ages, sin, cos, norm_w, W_in, W_out)
    res = run_bass_kernel_spmd(nc, in_maps, list(range(NCORES)), trace=trace)
    # results[i]["out"][c] holds reduced rows [c*TC + i*(TC/8) : +TC/8]
    nchunk = T // TC
    seg = TC // NCORES
    out = np.empty((T, HID), np.float32)
    for i in range(NCORES):
        oi = np.asarray(res.results[i]["out"], np.float32)
        for c in range(nchunk):
            r0 = c * TC + i * seg
            out[r0:r0 + seg] = oi[c]
    return out, res


def kernel(x, normed_ages, sin, cos, norm_w, W_in, W_out):
    out, _ = run(x, normed_ages, sin, cos, norm_w, W_in, W_out)
    return out
